# revision 7
# baseline (speedup 1.0000x reference)
"""Biased matrix-factorization batch scoring on 8 Trainium2 NeuronCores.

out[b] = 3.5 + user_biases[users[b]] + item_biases[items[b]]
         + dot(user_factors[users[b]], item_factors[items[b]])

Strategy: data-parallel over the batch (B=16384 -> 2048 per core), embedding
tables replicated to every core's HBM.  On the host the bias column is
concatenated onto each factor table (user row = [uf | ub | 1], item row =
[itf | 1 | ib], width 66) and the two tables are stacked into one combined
table (item rows offset by NUM_USERS), so the row-wise dot of a gathered
user row and item row yields factors-dot + both biases in one reduction.

Per core: one 8KB index DMA, 32 indirect-DMA gathers (128 rows x 264B each;
the hardware INDIRECT1D op gathers one row per partition), one elementwise
multiply of the user half with the item half, a grouped sum-reduction, +3.5,
and one 8KB store.  Raw Bass (no Tile) with a single semaphore per stage;
only the last gather carries a completion semaphore (single SWDGE queue ->
per-engine FIFO makes earlier gathers complete-before it).
"""

import numpy as np

GLOBAL_AVERAGE = 3.5
NUM_USERS = 1_000_000
NUM_ITEMS = 100_000
F = 64
B = 16384
NCORES = 8
BC = B // NCORES  # 2048 batch elements per core
P = 128
G = BC // P  # 16 batch elements per partition
W = F + 2  # 66: factors + bias column + ones column
NROW = 2 * G  # 32 gathered rows per partition (16 user + 16 item)

_BUILD_CACHE = {}


def build_nc(num_users=NUM_USERS, num_items=NUM_ITEMS, w=W):
    """Build + finalize the single-core Bass program (SPMD: same program on
    all 8 cores, each fed its own 2048-element index chunk)."""
    key = (num_users, num_items, w)
    if key in _BUILD_CACHE:
        return _BUILD_CACHE[key]

    import concourse.bass as bass
    import concourse.mybir as mybir
    from concourse.bass import IndirectOffsetOnAxis

    ncat = num_users + num_items
    nc = bass.Bass()
    idx = nc.dram_tensor("idx", [BC * 2], mybir.dt.int32, kind="ExternalInput")
    cat = nc.dram_tensor("cat", [ncat, w], mybir.dt.float32, kind="ExternalInput")
    out = nc.dram_tensor("out", [BC], mybir.dt.float32, kind="ExternalOutput")

    with (
        nc.sbuf_tensor([P, NROW], mybir.dt.int32) as t_idx,
        nc.sbuf_tensor([P, NROW * w], mybir.dt.float32) as rows,
        nc.sbuf_tensor([P, G * w], mybir.dt.float32) as prod,
        nc.sbuf_tensor([P, G], mybir.dt.float32) as res,
        nc.semaphore() as s_idx,
        nc.semaphore() as s_g,
        nc.semaphore() as s_c,
        nc.semaphore() as s_v,
        nc.semaphore() as s_o,
        nc.Block() as block,
    ):

        @block.sync
        def _(sync):
            # t_idx[p, j]: j<G -> user id for batch elem p*G+j; j>=G ->
            # num_users + item id for batch elem p*G+(j-G)
            sync.dma_start(
                t_idx[:], idx[:].rearrange("(p j) -> p j", j=NROW)
            ).then_inc(s_idx, 16)
            sync.wait_ge(s_c, 1)
            sync.dma_start(
                out[:].rearrange("(p g) -> p g", g=G), res[:]
            ).then_inc(s_o, 16)
            sync.wait_ge(s_o, 16)

        @block.gpsimd
        def _(g):
            g.wait_ge(s_idx, 16)
            for k in range(NROW):
                g.indirect_dma_start(
                    out=rows[:, k * w : (k + 1) * w],
                    out_offset=None,
                    in_=cat[:],
                    in_offset=IndirectOffsetOnAxis(ap=t_idx[:, k : k + 1], axis=0),
                ).then_inc(s_g, 16)

        @block.vector
        def _(v):
            v.wait_ge(s_g, NROW * 16)
            v.tensor_mul(prod[:], rows[:, : G * w], rows[:, G * w :]).then_inc(s_v, 1)
            v.wait_ge(s_v, 1)
            v.reduce_sum(
                res[:],
                prod[:].rearrange("p (g w) -> p g w", w=w),
                axis=mybir.AxisListType.X,
            ).then_inc(s_v, 1)
            v.wait_ge(s_v, 2)
            v.tensor_scalar_add(res[:], res[:], GLOBAL_AVERAGE).then_inc(s_c, 1)

    nc.finalize()
    _BUILD_CACHE[key] = nc
    return nc


def make_cat(user_factors, item_factors, user_biases, item_biases):
    """Combined table: row u = [uf[u] | ub[u] | 1]; row num_users+i =
    [itf[i] | 1 | ib[i]]."""
    nu, f = user_factors.shape
    ni = item_factors.shape[0]
    w = f + 2
    cat = np.empty((nu + ni, w), np.float32)
    cat[:nu, :f] = user_factors
    cat[:nu, f] = np.asarray(user_biases).reshape(nu)
    cat[:nu, f + 1] = 1.0
    cat[nu:, :f] = item_factors
    cat[nu:, f] = 1.0
    cat[nu:, f + 1] = np.asarray(item_biases).reshape(ni)
    return cat


def make_idx(users, items, num_users, ncores=NCORES):
    """Per-core interleaved index arrays matching the kernel's SBUF layout:
    flat[p*32 + j] = users[c*2048 + p*16 + j]          for j < 16
                   = num_users + items[c*2048 + p*16 + j-16]  otherwise."""
    u = np.asarray(users, dtype=np.int32).reshape(ncores, P, G)
    it = np.asarray(items, dtype=np.int32).reshape(ncores, P, G) + np.int32(num_users)
    inter = np.concatenate([u, it], axis=2)  # [ncores, P, 2G]
    return np.ascontiguousarray(inter.reshape(ncores, 2 * BC))


def kernel(users, items, user_factors, item_factors, user_biases, item_biases):
    from concourse.bass_utils import run_bass_kernel_spmd

    nc = build_nc()
    cat = make_cat(user_factors, item_factors, user_biases, item_biases)
    idx = make_idx(users, items, NUM_USERS)
    in_maps = [{"idx": idx[c], "cat": cat} for c in range(NCORES)]
    res = run_bass_kernel_spmd(nc, in_maps, core_ids=list(range(NCORES)))
    return np.concatenate([res.results[c]["out"] for c in range(NCORES)])


# revision 8
# speedup vs baseline: 1.0097x; 1.0097x over previous
"""Biased matrix-factorization batch scoring on 8 Trainium2 NeuronCores.

out[b] = 3.5 + user_biases[users[b]] + item_biases[items[b]]
         + dot(user_factors[users[b]], item_factors[items[b]])

Strategy: data-parallel over the batch (B=16384 -> 2048 per core), embedding
tables replicated to every core's HBM.  On the host the bias column is
concatenated onto each factor table (user row = [uf | ub | 1], item row =
[itf | 1 | ib], width 66) and the two tables are stacked into one combined
table (item rows offset by NUM_USERS), so the row-wise dot of a gathered
user row and item row yields factors-dot + both biases in one reduction.

Per core (raw Bass, manual semaphores): one 8KB index DMA, then 32
INDIRECT1D gathers on the GpSimd SWDGE (the hardware op gathers exactly one
random 264B row per partition, ~1.4us each -- the dominant cost), an
elementwise multiply of user rows with item rows (split in two halves that
overlap the second half of the gathers), a grouped sum-reduction, +3.5, and
one 8KB store.  A post-finalize pass strips the per-semaphore clear storm
the framework appends for semaphores this program never uses."""

import numpy as np

GLOBAL_AVERAGE = 3.5
NUM_USERS = 1_000_000
NUM_ITEMS = 100_000
F = 64
B = 16384
NCORES = 8
BC = B // NCORES
P = 128
G = BC // P  # 16
W = F + 2  # 66
NROW = 2 * G  # 32
HALF = NROW // 2  # 16 gathers per half (8 user + 8 item rows per partition)

_BUILD_CACHE = {}


def build_nc(num_users=NUM_USERS, num_items=NUM_ITEMS, w=W):
    key = (num_users, num_items, w)
    if key in _BUILD_CACHE:
        return _BUILD_CACHE[key]

    import concourse.bass as bass
    import concourse.mybir as mybir
    from concourse.bass import IndirectOffsetOnAxis

    ncat = num_users + num_items
    nc = bass.Bass()
    idx = nc.dram_tensor("idx", [BC * 2], mybir.dt.int32, kind="ExternalInput")
    cat = nc.dram_tensor("cat", [ncat, w], mybir.dt.float32, kind="ExternalInput")
    out = nc.dram_tensor("out", [BC], mybir.dt.float32, kind="ExternalOutput")

    # Half h covers batch elements p*G + [h*8, h*8+8): slots j in [h*16, h*16+16)
    # j%16 < 8 -> user row, else item row (for element p*G + h*8 + j%8).
    with (
        nc.sbuf_tensor([P, NROW], mybir.dt.int32) as t_idx,
        nc.sbuf_tensor([P, NROW * w], mybir.dt.float32) as rows,
        nc.sbuf_tensor([P, G * w], mybir.dt.float32) as prod,
        nc.sbuf_tensor([P, G], mybir.dt.float32) as res,
        nc.semaphore() as s_idx,
        nc.semaphore() as s_g,
        nc.semaphore() as s_g2,
        nc.semaphore() as s_c,
        nc.semaphore() as s_v,
        nc.semaphore() as s_o,
        nc.Block() as block,
    ):

        @block.gpsimd
        def _(g):
            g.dma_start(
                t_idx[:], idx[:].rearrange("(p j) -> p j", j=NROW)
            ).then_inc(s_idx, 16)
            g.wait_ge(s_idx, 16)
            for k in range(NROW):
                g.indirect_dma_start(
                    out=rows[:, k * w : (k + 1) * w],
                    out_offset=None,
                    in_=cat[:],
                    in_offset=IndirectOffsetOnAxis(ap=t_idx[:, k : k + 1], axis=0),
                ).then_inc(s_g if k < HALF else s_g2, 16)

        @block.vector
        def _(v):
            for h in range(2):
                lo = h * HALF * w
                v.wait_ge(s_g if h == 0 else s_g2, HALF * 16)
                v.tensor_mul(
                    prod[:, h * 8 * w : (h + 1) * 8 * w],
                    rows[:, lo : lo + 8 * w],
                    rows[:, lo + 8 * w : lo + 16 * w],
                ).then_inc(s_v, 1)
            v.wait_ge(s_v, 2)
            v.reduce_sum(
                res[:],
                prod[:].rearrange("p (g w) -> p g w", w=w),
                axis=mybir.AxisListType.X,
            ).then_inc(s_v, 1)
            v.wait_ge(s_v, 3)
            v.tensor_scalar_add(res[:], res[:], GLOBAL_AVERAGE).then_inc(s_c, 1)

        @block.sync
        def _(sync):
            sync.wait_ge(s_c, 1)
            sync.dma_start(
                out[:].rearrange("(p g) -> p g", g=G), res[:]
            ).then_inc(s_o, 16)
            sync.wait_ge(s_o, 16)

    nc.finalize()

    # Tail surgery: drop sem-clear EVENT_SEMAPHOREs for semaphores this
    # program never touches (the finalize postamble clears the whole dynamic
    # range one instruction at a time).
    used = set()
    for bb in nc.m.functions[0].blocks:
        for ins in bb.instructions:
            si = ins.sync_info
            if si:
                for u in list(si.on_update or []) + list(si.on_wait or []):
                    sid = getattr(u, "id", None)
                    if sid is not None:
                        used.add(sid)
    import concourse.mybir as mybir_

    for bb in nc.m.functions[0].blocks:
        keep = []
        for ins in bb.instructions:
            drop = False
            if type(ins).__name__ == "InstEventSemaphore":
                si = ins.sync_info
                ups = list(si.on_update or []) if si else []
                ws = list(si.on_wait or []) if si else []
                if not ws and len(ups) == 1:
                    u = ups[0]
                    if (
                        getattr(u, "value", None) == 0
                        and getattr(u, "sem_op", None) in ("set", "assign", None)
                        and getattr(u, "id", -1) not in used
                    ):
                        drop = True
            if not drop:
                keep.append(ins)
        if len(keep) != len(bb.instructions):
            bb.instructions[:] = keep

    _BUILD_CACHE[key] = nc
    return nc


def make_cat(user_factors, item_factors, user_biases, item_biases):
    nu, f = user_factors.shape
    ni = item_factors.shape[0]
    w = f + 2
    cat = np.empty((nu + ni, w), np.float32)
    cat[:nu, :f] = user_factors
    cat[:nu, f] = np.asarray(user_biases).reshape(nu)
    cat[:nu, f + 1] = 1.0
    cat[nu:, :f] = item_factors
    cat[nu:, f] = 1.0
    cat[nu:, f + 1] = np.asarray(item_biases).reshape(ni)
    return cat


def make_idx(users, items, num_users, ncores=NCORES):
    """flat[p*32 + h*16 + j] = user id of elem p*16+h*8+j     (j<8)
                             = num_users + item id of elem p*16+h*8+j-8 (j>=8)"""
    u = np.asarray(users, dtype=np.int32).reshape(ncores, P, 2, 8)
    it = np.asarray(items, dtype=np.int32).reshape(ncores, P, 2, 8) + np.int32(
        num_users
    )
    inter = np.concatenate([u, it], axis=3)  # [ncores, P, 2, 16] (h, u8|i8)
    return np.ascontiguousarray(inter.reshape(ncores, 2 * BC))


def kernel(users, items, user_factors, item_factors, user_biases, item_biases):
    from concourse.bass_utils import run_bass_kernel_spmd

    nc = build_nc()
    cat = make_cat(user_factors, item_factors, user_biases, item_biases)
    idx = make_idx(users, items, NUM_USERS)
    in_maps = [{"idx": idx[c], "cat": cat} for c in range(NCORES)]
    res = run_bass_kernel_spmd(nc, in_maps, core_ids=list(range(NCORES)))
    return np.concatenate([res.results[c]["out"] for c in range(NCORES)])


# revision 9
# speedup vs baseline: 1.0969x; 1.0864x over previous
"""Biased matrix-factorization batch scoring on 8 Trainium2 NeuronCores.

out[b] = 3.5 + user_biases[users[b]] + item_biases[items[b]]
         + dot(user_factors[users[b]], item_factors[items[b]])

Data-parallel over the batch (2048 elements per core), tables replicated in
every core's HBM.  Host packs both tables into one combined table of 66-wide
rows (user row = [uf | ub | 1], item row = [itf | 1 | ib + 3.5], items
offset by NUM_USERS) so the row-wise dot of the two gathered rows IS the
final answer.  Per core (raw Bass): one 8KB index DMA, 32 INDIRECT1D
gathers on the GpSimd SWDGE (one random 264B row per partition per op,
~1.4us each -- the hardware floor and the dominant cost), elementwise
multiply + grouped sum-reduction split in quarters that overlap the
gathers, one 8KB store."""

import numpy as np

GLOBAL_AVERAGE = 3.5
NUM_USERS = 1_000_000
NUM_ITEMS = 100_000
F = 64
B = 16384
NCORES = 8
BC = B // NCORES
P = 128
G = BC // P  # 16
W = F + 2  # 66
NROW = 2 * G  # 32 gathers
NQ = 4  # compute quarters
R = G // NQ  # 4 elements per quarter per partition

_BUILD_CACHE = {}


def build_nc(num_users=NUM_USERS, num_items=NUM_ITEMS, w=W):
    key = (num_users, num_items, w)
    if key in _BUILD_CACHE:
        return _BUILD_CACHE[key]

    import concourse.bass as bass
    import concourse.mybir as mybir
    from concourse.bass import IndirectOffsetOnAxis

    ncat = num_users + num_items
    nc = bass.Bass()
    idx = nc.dram_tensor("idx", [BC * 2], mybir.dt.int32, kind="ExternalInput")
    cat = nc.dram_tensor("cat", [ncat, w], mybir.dt.float32, kind="ExternalInput")
    out = nc.dram_tensor("out", [BC], mybir.dt.float32, kind="ExternalOutput")

    # Slot j in [q*8, q*8+8): j%8 < 4 -> user row of element p*16+q*4+(j%4),
    # else item row of the same element.  Gather op k handles slot column k.
    with (
        nc.sbuf_tensor([P, NROW], mybir.dt.int32) as t_idx,
        nc.sbuf_tensor([P, NROW * w], mybir.dt.float32) as rows,
        nc.sbuf_tensor([P, G * w], mybir.dt.float32) as prod,
        nc.sbuf_tensor([P, G], mybir.dt.float32) as res,
        nc.semaphore() as s_idx,
        nc.semaphore() as s_q0,
        nc.semaphore() as s_q1,
        nc.semaphore() as s_q2,
        nc.semaphore() as s_q3,
        nc.semaphore() as s_v,
        nc.semaphore() as s_c,
        nc.semaphore() as s_o,
        nc.Block() as block,
    ):
        s_q = [s_q0, s_q1, s_q2, s_q3]

        @block.sync
        def _(sync):
            sync.dma_start(
                t_idx[:], idx[:].rearrange("(p j) -> p j", j=NROW)
            ).then_inc(s_idx, 16)
            sync.wait_ge(s_c, NQ)
            sync.dma_start(
                out[:].rearrange("(p g) -> p g", g=G), res[:]
            ).then_inc(s_o, 16)
            sync.wait_ge(s_o, 16)

        @block.gpsimd
        def _(g):
            g.wait_ge(s_idx, 16)
            for k in range(NROW):
                g.indirect_dma_start(
                    out=rows[:, k * w : (k + 1) * w],
                    out_offset=None,
                    in_=cat[:],
                    in_offset=IndirectOffsetOnAxis(ap=t_idx[:, k : k + 1], axis=0),
                ).then_inc(s_q[k // (2 * R)], 16)

        @block.vector
        def _(v):
            for q in range(NQ):
                lo = q * 2 * R * w
                v.wait_ge(s_q[q], 2 * R * 16)
                v.tensor_mul(
                    prod[:, q * R * w : (q + 1) * R * w],
                    rows[:, lo : lo + R * w],
                    rows[:, lo + R * w : lo + 2 * R * w],
                ).then_inc(s_v, 1)
                v.wait_ge(s_v, q + 1)
                v.reduce_sum(
                    res[:, q * R : (q + 1) * R],
                    prod[:, q * R * w : (q + 1) * R * w].rearrange(
                        "p (g w) -> p g w", w=w
                    ),
                    axis=mybir.AxisListType.X,
                ).then_inc(s_c, 1)

    nc.finalize()

    # Tail surgery: drop sem-clear EVENT_SEMAPHOREs for semaphores this
    # program never touches.
    used = set()
    for bb in nc.m.functions[0].blocks:
        for ins in bb.instructions:
            si = ins.sync_info
            if si:
                for u in list(si.on_update or []) + list(si.on_wait or []):
                    sid = getattr(u, "id", None)
                    if sid is not None:
                        used.add(sid)
    for bb in nc.m.functions[0].blocks:
        keep = []
        for ins in bb.instructions:
            drop = False
            if type(ins).__name__ == "InstEventSemaphore":
                si = ins.sync_info
                ups = list(si.on_update or []) if si else []
                ws = list(si.on_wait or []) if si else []
                if not ws and len(ups) == 1:
                    u = ups[0]
                    if (
                        getattr(u, "value", None) == 0
                        and getattr(u, "sem_op", None) in ("set", "assign", None)
                        and getattr(u, "id", -1) not in used
                    ):
                        drop = True
            if not drop:
                keep.append(ins)
        if len(keep) != len(bb.instructions):
            bb.instructions[:] = keep

    _BUILD_CACHE[key] = nc
    return nc


def make_cat(user_factors, item_factors, user_biases, item_biases):
    """Row u = [uf[u] | ub[u] | 1]; row num_users+i = [itf[i] | 1 | ib[i]+3.5]
    so the row-wise dot alone is the final answer."""
    nu, f = user_factors.shape
    ni = item_factors.shape[0]
    w = f + 2
    cat = np.empty((nu + ni, w), np.float32)
    cat[:nu, :f] = user_factors
    cat[:nu, f] = np.asarray(user_biases).reshape(nu)
    cat[:nu, f + 1] = 1.0
    cat[nu:, :f] = item_factors
    cat[nu:, f] = 1.0
    cat[nu:, f + 1] = np.asarray(item_biases).reshape(ni) + np.float32(GLOBAL_AVERAGE)
    return cat


def make_idx(users, items, num_users, ncores=NCORES):
    """flat[p*32 + q*8 + j] = user id of element p*16+q*4+j        (j<4)
                            = num_users + item id of elem p*16+q*4+j-4 (j>=4)"""
    u = np.asarray(users, dtype=np.int32).reshape(ncores, P, NQ, R)
    it = np.asarray(items, dtype=np.int32).reshape(ncores, P, NQ, R) + np.int32(
        num_users
    )
    inter = np.concatenate([u, it], axis=3)  # [ncores, P, NQ, 2R]
    return np.ascontiguousarray(inter.reshape(ncores, 2 * BC))


def kernel(users, items, user_factors, item_factors, user_biases, item_biases):
    from concourse.bass_utils import run_bass_kernel_spmd

    nc = build_nc()
    cat = make_cat(user_factors, item_factors, user_biases, item_biases)
    idx = make_idx(users, items, NUM_USERS)
    in_maps = [{"idx": idx[c], "cat": cat} for c in range(NCORES)]
    res = run_bass_kernel_spmd(nc, in_maps, core_ids=list(range(NCORES)))
    return np.concatenate([res.results[c]["out"] for c in range(NCORES)])
